# revision 6
# baseline (speedup 1.0000x reference)
"""Trainium2 Bass kernel for 16-head causal MHA (B=2, S=2048, D=1024), fp32.

Sharding (8 cores): batch x head-group. Core c handles batch c//4 and heads
4*(c%4)..4*(c%4)+3 (D columns 256*(c%4) .. +256). QKV weights column-sharded,
Wo row-sharded (Megatron). Per-core partial outputs are summed on the host
(the row-parallel all-reduce), bo added once per batch.

x is pre-transposed on the host (xt = x[b].T, [D, S]) so no PE transposes
are needed: xT tiles DMA straight into [d-part, tok] layout.

Per-core dataflow (all matmuls in float32r: ~12-bit mantissa, full speed):
  phase 1 (per 512-token chunk): DMA xT -> qT/kT = W.T @ xT
           ([head-dim part, tok free]) + bias via DVE eviction;
           v = xT.T @ Wv directly in [tok part, hd free] (xt tiles are the
           stationary operand), bias via a rank-1 ones x bv matmul folded
           into the same PSUM accumulation; ones column lives at col 64.
  phase 2 (per q-superblock i, head-pair p, k-block j<=4i+3):
           S_T[tk,tq] = k @ qT (2 heads packed in PE row groups);
           expS = exp(S_T) straight to f32r (no max subtraction; scores O(6));
           diagonal-band tiles: triangular mask-mul of the 128-col stripe on
           GPSIMD into a separate tile, so the wide part of the ctx matmul
           depends only on the ACT exp;
           ctxT[hd+1, tq] += [v|1].T @ expS  (ones row = softmax denominator);
           normalize via K=1 broadcast matmul + DVE reciprocal + GPSIMD mul.
  phase 3: out[tq, :] = ctxT_norm.T @ Wo_slice (partial), DMA out via SP.

Emission interleaves phase1(c+1) / phase3(c-1) units into phase2(c)'s
j-loop so PE fills its ACT-wait gaps with projection work.
"""

import numpy as np

import concourse.bacc as bacc
import concourse.mybir as mybir
import concourse.tile as tile
from concourse import bass_utils

F32 = mybir.dt.float32
F32R = mybir.dt.float32r

B, S, D = 2, 2048, 1024
H, HD = 16, 64
NCORES = 8
HPC = 4            # heads per core
DC = HPC * HD      # 256 D-columns per core
NPAIR = 2          # head pairs per core (128 partitions each)
QSB = 512          # q superblock
KB = 128           # k block
NKT = S // KB      # 16 k tiles
NCH = S // QSB     # 4 chunks / q superblocks


def build(loop_n=None):
    nc = bacc.Bacc("TRN2", target_bir_lowering=False, debug=False)

    xt_d = nc.dram_tensor("xt", [D, S], F32R, kind="ExternalInput")
    wq = nc.dram_tensor("wq", [D, DC], F32R, kind="ExternalInput")
    wk = nc.dram_tensor("wk", [D, DC], F32R, kind="ExternalInput")
    wv = nc.dram_tensor("wv", [D, DC], F32R, kind="ExternalInput")
    wo = nc.dram_tensor("wo", [DC, D], F32R, kind="ExternalInput")
    bq = nc.dram_tensor("bq", [DC], F32, kind="ExternalInput")
    bk = nc.dram_tensor("bk", [DC], F32, kind="ExternalInput")
    bv = nc.dram_tensor("bv", [DC], F32R, kind="ExternalInput")
    out = nc.dram_tensor("out", [S, D], F32, kind="ExternalOutput")

    KT = D // 128  # 8 contraction k-tiles for projections

    with tile.TileContext(nc) as tc:
        with tc.tile_pool(name="persist", bufs=1) as pp, \
             tc.tile_pool(name="wpool", bufs=1) as wp:
            # ---- constants / weights ----
            scratch = pp.tile([128, 128], F32)
            nc.vector.memset(scratch, 1.0)

            # triangular stripe mask: keep col >= row (causal, incl diagonal)
            tri_f = pp.tile([128, 128], F32)
            nc.gpsimd.memset(tri_f, 1.0)
            nc.gpsimd.affine_select(
                out=tri_f, in_=tri_f,
                compare_op=mybir.AluOpType.is_ge, fill=0.0,
                base=0, pattern=[[1, 128]], channel_multiplier=-1)
            tri = pp.tile([128, 128], F32R)
            nc.vector.tensor_copy(tri, tri_f)

            ones_col65 = pp.tile([65, 64], F32R)
            nc.vector.tensor_copy(ones_col65[64:65, :], scratch[0:1, 0:64])
            ones_col = ones_col65[64:65, :]   # base partition 64, matches sums row

            wq_sb = wp.tile([128, KT, DC], F32R)
            wk_sb = wp.tile([128, KT, DC], F32R)
            wv_sb = wp.tile([128, KT, DC], F32R)
            bq_sb = pp.tile([128, NPAIR], F32)
            bk_sb = pp.tile([128, NPAIR], F32)
            bv_row = pp.tile([1, DC], F32R)
            nc.scalar.dma_start(out=bq_sb, in_=bq.ap().rearrange("(t p) -> p t", p=128))
            nc.scalar.dma_start(out=bk_sb, in_=bk.ap().rearrange("(t p) -> p t", p=128))
            nc.scalar.dma_start(out=bv_row, in_=bv.ap().rearrange("(o d) -> o d", o=1))
            nc.scalar.dma_start(out=wq_sb, in_=wq.ap().rearrange("(t p) n -> p t n", p=128))
            nc.scalar.dma_start(out=wk_sb, in_=wk.ap().rearrange("(t p) n -> p t n", p=128))
            nc.scalar.dma_start(out=wv_sb, in_=wv.ap().rearrange("(t p) n -> p t n", p=128))
            wo_sb = pp.tile([128, NPAIR, D], F32R)
            nc.scalar.dma_start(out=wo_sb, in_=wo.ap().rearrange("(t p) n -> p t n", p=128))

            # ---- persistent activations ----
            qT = pp.tile([128, NPAIR, S], F32R)       # [hd-pair part, pair, tok]
            kT = pp.tile([128, NPAIR, S], F32R)
            v_all = pp.tile([128, NKT, HPC, 65], F32R)  # [tk part, tktile, head, hd|1]
            ctxT = pp.tile([128, NPAIR, S], F32R)

            # ones columns of v_all (col 64 of every (tktile, head) slot)
            nc.vector.tensor_copy(
                v_all[:, :, :, 64].rearrange("p a b -> p (a b)"),
                scratch[:, 0:NKT * HPC])

            # PE warm-up: ~4us of dummy matmuls during the initial DMA wait
            # window so the HAM clock-gate is at 8/8 when real work arrives.
            warm_sb = pp.tile([128, 128], F32)

            with tc.tile_pool(name="xt", bufs=2) as xt_p, \
                 tc.tile_pool(name="expp", bufs=4) as expp, \
                 tc.tile_pool(name="exm", bufs=4) as exm_p, \
                 tc.tile_pool(name="rec", bufs=2) as rec_p, \
                 tc.tile_pool(name="outp", bufs=2) as out_p, \
                 tc.tile_pool(name="ps_u", bufs=2, space="PSUM") as ps_u, \
                 tc.tile_pool(name="ps_s", bufs=2, space="PSUM") as ps_s, \
                 tc.tile_pool(name="ps_c", bufs=2, space="PSUM") as ps_c:

                pwarm = ps_u.tile([128, 512], F32, name="pu")
                for w in range(36):
                    nc.tensor.matmul(pwarm[:, 0:128], tri, tri[:, 0:128],
                                     start=(w == 0), stop=(w == 35))
                nc.vector.tensor_copy(warm_sb, pwarm[:, 0:128])

                def p1_units(c):
                    """projections for tokens [512c, 512c+512); yields per unit."""
                    xt = xt_p.tile([128, KT, QSB], F32R, name="xt")
                    if c == 0:  # split so the first matmuls start sooner
                        for a in range(4):
                            nc.sync.dma_start(
                                out=xt[:, 2 * a:2 * a + 2, :],
                                in_=xt_d.ap()[a * 256:(a + 1) * 256,
                                              c * QSB:(c + 1) * QSB].rearrange(
                                    "(t p) s -> p t s", p=128))
                    else:
                        for a in range(2):
                            nc.sync.dma_start(
                                out=xt[:, 4 * a:4 * a + 4, :],
                                in_=xt_d.ap()[a * 512:(a + 1) * 512,
                                              c * QSB:(c + 1) * QSB].rearrange(
                                    "(t p) s -> p t s", p=128))
                    for p in range(NPAIR):
                        for (w_sb, b_sb, dstT) in ((wq_sb, bq_sb, qT), (wk_sb, bk_sb, kT)):
                            pq = ps_u.tile([128, 512], F32, name="pu")
                            for kt in range(KT):
                                nc.tensor.matmul(
                                    pq, w_sb[:, kt, p * 128:(p + 1) * 128],
                                    xt[:, kt, :],
                                    start=(kt == 0), stop=(kt == KT - 1))
                            nc.vector.tensor_scalar_add(
                                dstT[:, p, c * QSB:(c + 1) * QSB], pq,
                                b_sb[:, p:p + 1])
                            yield
                    for tt in range(4):  # v in natural [tok, hd] layout
                        tkt = 4 * c + tt
                        pv = ps_u.tile([128, 512], F32, name="pu")
                        for kt in range(KT):
                            nc.tensor.matmul(
                                pv[:, 0:DC],
                                xt[:, kt, tt * 128:(tt + 1) * 128],
                                wv_sb[:, kt, :],
                                start=(kt == 0), stop=False)
                        nc.tensor.matmul(     # rank-1 bias add: ones.T @ bv
                            pv[:, 0:DC], tri[0:1, 0:128], bv_row,
                            start=False, stop=True)
                        nc.vector.tensor_copy(
                            v_all[:, tkt, :, 0:64],
                            pv[:, 0:DC].rearrange("p (h d) -> p h d", h=HPC))
                        yield

                def p2_units(i):
                    """attention for tq in [512i, 512i+512); yields per (pair, j)."""
                    nj = 4 * i + 4

                    def emit_norm(p, pctx):
                        # evict raw pctx to SBUF fast (frees the PSUM
                        # accumulator), then normalize
                        for hp in range(2):
                            cu = rec_p.tile([65, QSB], F32R, name="cu")
                            nc.vector.tensor_copy(cu, pctx[hp])
                            sums = cu[64:65, :]
                            pbc = ps_s.tile([128, 2, QSB], F32, name="ps")
                            nc.tensor.matmul(pbc[0:64, 0, :], ones_col, sums,
                                             start=True, stop=True)
                            rec = rec_p.tile([64, QSB], F32, name="rec")
                            nc.vector.reciprocal(rec, pbc[0:64, 0, :])
                            hlo = hp * 64
                            nc.gpsimd.tensor_tensor(
                                ctxT[hlo:hlo + 64, p, i * QSB:(i + 1) * QSB],
                                cu[0:64, :], rec, op=mybir.AluOpType.mult)

                    norm_q = []
                    for p in range(NPAIR):
                        pctx = [ps_c.tile([65, QSB], F32, name="pctx")
                                for _ in range(2)]
                        for j in range(nj):
                            r = j - 4 * i
                            lo = 128 * r
                            lo_col = max(0, lo)  # live columns start
                            ps = ps_s.tile([128, 2, QSB], F32, name="ps")
                            for hp in range(2):  # head in pair: PE row groups
                                hlo = hp * 64
                                nc.tensor.matmul(
                                    ps[:, hp, lo_col:],
                                    kT[hlo:hlo + 64, p, j * KB:(j + 1) * KB],
                                    qT[hlo:hlo + 64, p, i * QSB + lo_col:(i + 1) * QSB],
                                    start=True, stop=True,
                                    tile_position=(hlo, 0))
                            ex = expp.tile([128, 2, QSB], F32R, name="ex")
                            nc.scalar.activation(
                                ex[:, :, lo_col:], ps[:, :, lo_col:],
                                mybir.ActivationFunctionType.Exp)
                            if r >= 0:
                                # diagonal band: mask the 128-col stripe into a
                                # separate tile on GPSIMD (off the ACT->ctx path)
                                exm = exm_p.tile([128, 2, 128], F32R, name="exm")
                                import concourse.bass as bass_mod
                                tri_b = bass_mod.AP(
                                    tensor=tri.tensor, offset=tri.offset,
                                    ap=[tri.ap[0], [0, 2], tri.ap[1]])
                                nc.gpsimd.tensor_tensor(
                                    exm, ex[:, :, lo:lo + 128], tri_b,
                                    op=mybir.AluOpType.mult)
                                if lo + 128 < QSB:  # unmasked wide parts first
                                    for hp in range(2):
                                        nc.tensor.matmul(
                                            pctx[hp][:, lo + 128:],
                                            v_all[:, j, 2 * p + hp, :],
                                            ex[:, hp, lo + 128:],
                                            start=(j == 0), stop=False)
                                for hp in range(2):
                                    nc.tensor.matmul(
                                        pctx[hp][:, lo:lo + 128],
                                        v_all[:, j, 2 * p + hp, :],
                                        exm[:, hp, :],
                                        start=False, stop=(j == nj - 1))
                            else:
                                for hp in range(2):
                                    nc.tensor.matmul(
                                        pctx[hp],
                                        v_all[:, j, 2 * p + hp, :],
                                        ex[:, hp, :],
                                        start=(j == 0), stop=False)
                            yield
                            if norm_q:  # prev pair's norm, one j of lookahead
                                emit_norm(*norm_q.pop())
                        norm_q.append((p, pctx))
                    for item in norm_q:
                        emit_norm(*item)
                    yield

                def p3_units(i):
                    """output projection for tq in [512i, 512i+512)."""
                    for tpair in range(2):
                        ob = out_p.tile([128, 2, D], F32, name="ob")
                        for t2 in range(2):
                            tt = 4 * i + 2 * tpair + t2
                            for d in range(2):
                                po = ps_u.tile([128, 512], F32, name="pu")
                                for p in range(NPAIR):
                                    nc.tensor.matmul(
                                        po, ctxT[:, p, tt * 128:(tt + 1) * 128],
                                        wo_sb[:, p, d * QSB:(d + 1) * QSB],
                                        start=(p == 0), stop=(p == NPAIR - 1))
                                nc.any.tensor_copy(
                                    ob[:, t2, d * QSB:(d + 1) * QSB], po)
                                yield
                        tt0 = 4 * i + 2 * tpair
                        if i == NCH - 1 and tpair == 1:
                            # tail: split so the first half overlaps the
                            # second half's evictions
                            for t2 in range(2):
                                nc.sync.dma_start(
                                    out=out.ap()[(tt0 + t2) * 128:
                                                 (tt0 + t2 + 1) * 128, :],
                                    in_=ob[:, t2, :])
                        else:
                            nc.sync.dma_start(
                                out=out.ap()[tt0 * 128:(tt0 + 2) * 128, :].rearrange(
                                    "(t p) d -> p t d", p=128),
                                in_=ob)

                # interleaved emission: phase1(c+1) + phase3(c-1) ride along
                # phase2(c)'s j-loop so PE fills ACT-wait gaps.
                import contextlib
                loop_cm = tc.For_i(0, loop_n, 1) if loop_n else contextlib.nullcontext()
                with loop_cm:
                  for _ in p1_units(0):
                    pass
                  for c in range(NCH):
                      extras = []
                      if c + 1 < NCH:
                          extras.append(p1_units(c + 1))
                      if c > 0:
                          extras.append(p3_units(c - 1))

                      def drain_extras(k):
                          n = 0
                          while extras and n < k:
                              try:
                                  next(extras[0])
                                  n += 1
                              except StopIteration:
                                  extras.pop(0)

                      n2 = 2 * (4 * c + 4) + 2
                      nx = 8 + (8 if c + 1 < NCH else 0)
                      per = max(1, -(-nx // n2))
                      for _ in p2_units(c):
                          drain_extras(per)
                      drain_extras(1000)
                  for _ in p3_units(NCH - 1):
                      pass

    nc.compile()
    return nc


def prepare_in_maps(x, Wq, bq_, Wk, bk_, Wv, bv_, Wo, bo_):
    x = np.asarray(x, np.float32)
    xtc = [np.ascontiguousarray(x[0].T), np.ascontiguousarray(x[1].T)]
    in_maps = []
    for c in range(NCORES):
        b = c // 4
        g = c % 4
        sl = slice(DC * g, DC * (g + 1))
        in_maps.append({
            "xt": xtc[b],
            "wq": np.ascontiguousarray(np.asarray(Wq, np.float32)[:, sl] * 0.125),
            "wk": np.ascontiguousarray(np.asarray(Wk, np.float32)[:, sl]),
            "wv": np.ascontiguousarray(np.asarray(Wv, np.float32)[:, sl]),
            "wo": np.ascontiguousarray(np.asarray(Wo, np.float32)[sl, :]),
            "bq": np.ascontiguousarray(np.asarray(bq_, np.float32)[sl] * 0.125),
            "bk": np.ascontiguousarray(np.asarray(bk_, np.float32)[sl]),
            "bv": np.ascontiguousarray(np.asarray(bv_, np.float32)[sl]),
        })
    return in_maps


_NC_CACHE = {}


def _get_nc():
    if "nc" not in _NC_CACHE:
        _NC_CACHE["nc"] = build()
    return _NC_CACHE["nc"]


def kernel(x, Wq, bq, Wk, bk, Wv, bv, Wo, bo, _trace=False):
    nc = _get_nc()
    in_maps = prepare_in_maps(x, Wq, bq, Wk, bk, Wv, bv, Wo, bo)
    res = bass_utils.run_bass_kernel_spmd(
        nc, in_maps, core_ids=list(range(NCORES)), trace=_trace)
    if _trace:
        _NC_CACHE["last_results"] = res
    partials = [res.results[c]["out"] for c in range(NCORES)]
    bo = np.asarray(bo, np.float32)
    full = np.stack([
        partials[0] + partials[1] + partials[2] + partials[3] + bo,
        partials[4] + partials[5] + partials[6] + partials[7] + bo,
    ]).astype(np.float32)
    return full


# revision 11
# speedup vs baseline: 1.1494x; 1.1494x over previous
"""Trainium2 Bass kernel for 16-head causal MHA (B=2, S=2048, D=1024), fp32.

Sharding (8 cores): batch x head-group. Core c handles batch c//4 and heads
4*(c%4)..4*(c%4)+3 (D columns 256*(c%4) .. +256). QKV weights column-sharded,
Wo row-sharded (Megatron). Per-core partial outputs are summed on the host
(the row-parallel all-reduce), bo added once per batch.

x is pre-transposed on the host (xt = x[b].T, [D, S]) so no PE transposes
are needed: xT tiles DMA straight into [d-part, tok] layout.

Per-core dataflow (all matmuls in float32r: ~12-bit mantissa, full speed):
  phase 1 (per 512-token chunk): DMA xT -> qT/kT = W.T @ xT
           ([head-dim part, tok free]) + bias via DVE eviction;
           v = xT.T @ Wv directly in [tok part, hd free] (xt tiles are the
           stationary operand), bias via a rank-1 ones x bv matmul folded
           into the same PSUM accumulation; ones column lives at col 64.
  phase 2 (per q-superblock i, head-pair p, k-block j<=4i+3):
           S_T[tk,tq] = k @ qT (2 heads packed in PE row groups);
           expS = exp(S_T) straight to f32r (no max subtraction; scores O(6));
           diagonal-band tiles: triangular mask-mul of the 128-col stripe on
           GPSIMD into a separate tile, so the wide part of the ctx matmul
           depends only on the ACT exp;
           ctxT[hd+1, tq] += [v|1].T @ expS  (ones row = softmax denominator);
           normalize via K=1 broadcast matmul + DVE reciprocal + GPSIMD mul.
  phase 3: out[tq, :] = ctxT_norm.T @ Wo_slice (partial), DMA out via SP.

Emission interleaves phase1(c+1) / phase3(c-1) units into phase2(c)'s
j-loop so PE fills its ACT-wait gaps with projection work.
"""

import numpy as np

import concourse.bacc as bacc
import concourse.mybir as mybir
import concourse.tile as tile
from concourse import bass_utils

F32 = mybir.dt.float32
F32R = mybir.dt.float32r

B, S, D = 2, 2048, 1024
H, HD = 16, 64
NCORES = 8
HPC = 4            # heads per core
DC = HPC * HD      # 256 D-columns per core
NPAIR = 2          # head pairs per core (128 partitions each)
QSB = 512          # q superblock
KB = 128           # k block
NKT = S // KB      # 16 k tiles
NCH = S // QSB     # 4 chunks / q superblocks


def build(loop_n=None, unroll=1, staggered=False):
    nc = bacc.Bacc("TRN2", target_bir_lowering=False, debug=False)

    xt_d = nc.dram_tensor("xt", [D, S], F32R, kind="ExternalInput")
    wq = nc.dram_tensor("wq", [D, DC], F32R, kind="ExternalInput")
    wk = nc.dram_tensor("wk", [D, DC], F32R, kind="ExternalInput")
    wv = nc.dram_tensor("wv", [D, DC], F32R, kind="ExternalInput")
    wo = nc.dram_tensor("wo", [DC, D], F32R, kind="ExternalInput")
    bq = nc.dram_tensor("bq", [DC], F32, kind="ExternalInput")
    bk = nc.dram_tensor("bk", [DC], F32, kind="ExternalInput")
    bv = nc.dram_tensor("bv", [DC], F32R, kind="ExternalInput")
    out = nc.dram_tensor("out", [S, D], F32, kind="ExternalOutput")

    KT = D // 128  # 8 contraction k-tiles for projections

    with tile.TileContext(nc) as tc:
        with tc.tile_pool(name="persist", bufs=1) as pp, \
             tc.tile_pool(name="wpool", bufs=1) as wp:
            # ---- constants / weights ----
            scratch = pp.tile([128, 128], F32)
            nc.vector.memset(scratch, 1.0)

            # triangular stripe mask: keep col >= row (causal, incl diagonal)
            tri_f = pp.tile([128, 128], F32)
            nc.gpsimd.memset(tri_f, 1.0)
            nc.gpsimd.affine_select(
                out=tri_f, in_=tri_f,
                compare_op=mybir.AluOpType.is_ge, fill=0.0,
                base=0, pattern=[[1, 128]], channel_multiplier=-1)
            tri = pp.tile([128, 128], F32R)
            nc.vector.tensor_copy(tri, tri_f)

            ones_col65 = pp.tile([65, 64], F32R)
            nc.vector.tensor_copy(ones_col65[64:65, :], scratch[0:1, 0:64])
            ones_col = ones_col65[64:65, :]   # base partition 64, matches sums row

            wq_sb = wp.tile([128, KT, DC], F32R)
            wk_sb = wp.tile([128, KT, DC], F32R)
            wv_sb = wp.tile([128, KT, DC], F32R)
            bq_sb = pp.tile([128, NPAIR], F32)
            bk_sb = pp.tile([128, NPAIR], F32)
            bv_row = pp.tile([1, DC], F32R)
            nc.scalar.dma_start(out=bq_sb, in_=bq.ap().rearrange("(t p) -> p t", p=128))
            nc.scalar.dma_start(out=bk_sb, in_=bk.ap().rearrange("(t p) -> p t", p=128))
            nc.scalar.dma_start(out=bv_row, in_=bv.ap().rearrange("(o d) -> o d", o=1))
            nc.scalar.dma_start(out=wq_sb, in_=wq.ap().rearrange("(t p) n -> p t n", p=128))
            nc.scalar.dma_start(out=wk_sb, in_=wk.ap().rearrange("(t p) n -> p t n", p=128))
            nc.scalar.dma_start(out=wv_sb, in_=wv.ap().rearrange("(t p) n -> p t n", p=128))
            wo_sb = pp.tile([128, NPAIR, D], F32R)
            nc.scalar.dma_start(out=wo_sb, in_=wo.ap().rearrange("(t p) n -> p t n", p=128))

            # ---- persistent activations ----
            qT = pp.tile([128, NPAIR, S], F32R)       # [hd-pair part, pair, tok]
            kT = pp.tile([128, NPAIR, S], F32R)
            v_all = pp.tile([128, NKT, HPC, 65], F32R)  # [tk part, tktile, head, hd|1]
            ctxT = pp.tile([128, NPAIR, S], F32R)

            # ones columns of v_all (col 64 of every (tktile, head) slot)
            nc.vector.tensor_copy(
                v_all[:, :, :, 64].rearrange("p a b -> p (a b)"),
                scratch[:, 0:NKT * HPC])

            # PE warm-up: ~4us of dummy matmuls during the initial DMA wait
            # window so the HAM clock-gate is at 8/8 when real work arrives.
            warm_sb = pp.tile([128, 128], F32)

            with tc.tile_pool(name="xt", bufs=2) as xt_p, \
                 tc.tile_pool(name="expp", bufs=4) as expp, \
                 tc.tile_pool(name="exm", bufs=4) as exm_p, \
                 tc.tile_pool(name="rec", bufs=2) as rec_p, \
                 tc.tile_pool(name="outp", bufs=2) as out_p, \
                 tc.tile_pool(name="ps_u", bufs=2, space="PSUM") as ps_u, \
                 tc.tile_pool(name="ps_s", bufs=2, space="PSUM") as ps_s, \
                 tc.tile_pool(name="ps_c", bufs=2, space="PSUM") as ps_c:

                pwarm = ps_u.tile([128, 512], F32, name="pu")
                for w in range(36):
                    nc.tensor.matmul(pwarm[:, 0:128], tri, tri[:, 0:128],
                                     start=(w == 0), stop=(w == 35))
                nc.vector.tensor_copy(warm_sb, pwarm[:, 0:128])

                def p1_units(c):
                    """projections for tokens [512c, 512c+512); yields per unit."""
                    xt = xt_p.tile([128, KT, QSB], F32R, name="xt")
                    if c == 0:  # split so the first matmuls start sooner
                        for a in range(4):
                            nc.sync.dma_start(
                                out=xt[:, 2 * a:2 * a + 2, :],
                                in_=xt_d.ap()[a * 256:(a + 1) * 256,
                                              c * QSB:(c + 1) * QSB].rearrange(
                                    "(t p) s -> p t s", p=128))
                    else:
                        for a in range(2):
                            nc.sync.dma_start(
                                out=xt[:, 4 * a:4 * a + 4, :],
                                in_=xt_d.ap()[a * 512:(a + 1) * 512,
                                              c * QSB:(c + 1) * QSB].rearrange(
                                    "(t p) s -> p t s", p=128))
                    for p in range(NPAIR):
                        for (w_sb, b_sb, dstT) in ((wq_sb, bq_sb, qT), (wk_sb, bk_sb, kT)):
                            pq = ps_u.tile([128, 512], F32, name="pu")
                            for kt in range(KT):
                                nc.tensor.matmul(
                                    pq, w_sb[:, kt, p * 128:(p + 1) * 128],
                                    xt[:, kt, :],
                                    start=(kt == 0), stop=(kt == KT - 1))
                            nc.vector.tensor_scalar_add(
                                dstT[:, p, c * QSB:(c + 1) * QSB], pq,
                                b_sb[:, p:p + 1])
                            yield
                    for tt in range(4):  # v in natural [tok, hd] layout
                        tkt = 4 * c + tt
                        pv = ps_u.tile([128, 512], F32, name="pu")
                        for kt in range(KT):
                            nc.tensor.matmul(
                                pv[:, 0:DC],
                                xt[:, kt, tt * 128:(tt + 1) * 128],
                                wv_sb[:, kt, :],
                                start=(kt == 0), stop=False)
                        nc.tensor.matmul(     # rank-1 bias add: ones.T @ bv
                            pv[:, 0:DC], tri[0:1, 0:128], bv_row,
                            start=False, stop=True)
                        nc.vector.tensor_copy(
                            v_all[:, tkt, :, 0:64],
                            pv[:, 0:DC].rearrange("p (h d) -> p h d", h=HPC))
                        yield

                def p2_units(i):
                    """attention for tq in [512i, 512i+512); yields per (pair, j)."""
                    nj = 4 * i + 4

                    def emit_norm(p, pctx):
                        # evict raw pctx to SBUF fast (frees the PSUM
                        # accumulator), then normalize
                        for hp in range(2):
                            cu = rec_p.tile([65, QSB], F32R, name="cu")
                            nc.vector.tensor_copy(cu, pctx[hp])
                            sums = cu[64:65, :]
                            pbc = ps_s.tile([128, 2, QSB], F32, name="ps")
                            nc.tensor.matmul(pbc[0:64, 0, :], ones_col, sums,
                                             start=True, stop=True)
                            rec = rec_p.tile([64, QSB], F32, name="rec")
                            nc.vector.reciprocal(rec, pbc[0:64, 0, :])
                            hlo = hp * 64
                            nc.gpsimd.tensor_tensor(
                                ctxT[hlo:hlo + 64, p, i * QSB:(i + 1) * QSB],
                                cu[0:64, :], rec, op=mybir.AluOpType.mult)

                    def emit_ctx(p, pctx, j, ex, exm):
                        # ctx accumulation for k-block j (deferred one j so
                        # the next score matmuls sit ahead of it in PE FIFO)
                        r = j - 4 * i
                        lo = 128 * r
                        if r >= 0:
                            if lo + 128 < QSB:  # unmasked wide parts first
                                for hp in range(2):
                                    nc.tensor.matmul(
                                        pctx[hp][:, lo + 128:],
                                        v_all[:, j, 2 * p + hp, :],
                                        ex[:, hp, lo + 128:],
                                        start=(j == 0), stop=False)
                            for hp in range(2):
                                nc.tensor.matmul(
                                    pctx[hp][:, lo:lo + 128],
                                    v_all[:, j, 2 * p + hp, :],
                                    exm[:, hp, :],
                                    start=False, stop=(j == nj - 1))
                        else:
                            for hp in range(2):
                                nc.tensor.matmul(
                                    pctx[hp],
                                    v_all[:, j, 2 * p + hp, :],
                                    ex[:, hp, :],
                                    start=(j == 0), stop=False)

                    norm_q = []
                    prev = None  # pending ctx: (p, pctx, j, ex, exm)
                    for p in range(NPAIR):
                        pctx = [ps_c.tile([65, QSB], F32, name="pctx")
                                for _ in range(2)]
                        for j in range(nj):
                            r = j - 4 * i
                            lo = 128 * r
                            lo_col = max(0, lo)  # live columns start
                            ps = ps_s.tile([128, 2, QSB], F32, name="ps")
                            for hp in range(2):  # head in pair: PE row groups
                                hlo = hp * 64
                                nc.tensor.matmul(
                                    ps[:, hp, lo_col:],
                                    kT[hlo:hlo + 64, p, j * KB:(j + 1) * KB],
                                    qT[hlo:hlo + 64, p, i * QSB + lo_col:(i + 1) * QSB],
                                    start=True, stop=True,
                                    tile_position=(hlo, 0))
                            ex = expp.tile([128, 2, QSB], F32R, name="ex")
                            nc.scalar.activation(
                                ex[:, :, lo_col:], ps[:, :, lo_col:],
                                mybir.ActivationFunctionType.Exp)
                            exm = None
                            if r >= 0:
                                # diagonal band: mask the 128-col stripe into a
                                # separate tile on GPSIMD (off the ACT->ctx path)
                                exm = exm_p.tile([128, 2, 128], F32R, name="exm")
                                import concourse.bass as bass_mod
                                tri_b = bass_mod.AP(
                                    tensor=tri.tensor, offset=tri.offset,
                                    ap=[tri.ap[0], [0, 2], tri.ap[1]])
                                nc.gpsimd.tensor_tensor(
                                    exm, ex[:, :, lo:lo + 128], tri_b,
                                    op=mybir.AluOpType.mult)
                            if prev is not None:
                                emit_ctx(*prev)
                            prev = (p, pctx, j, ex, exm)
                            yield
                            if norm_q:  # prev pair's norm, one j of lookahead
                                emit_norm(*norm_q.pop())
                        norm_q.append((p, pctx))
                    if prev is not None:
                        emit_ctx(*prev)
                    for item in norm_q:
                        emit_norm(*item)
                    yield

                def p3_units(i):
                    """output projection for tq in [512i, 512i+512)."""
                    for tpair in range(2):
                        ob = out_p.tile([128, 2, D], F32, name="ob")
                        for t2 in range(2):
                            tt = 4 * i + 2 * tpair + t2
                            for d in range(2):
                                po = ps_u.tile([128, 512], F32, name="pu")
                                for p in range(NPAIR):
                                    nc.tensor.matmul(
                                        po, ctxT[:, p, tt * 128:(tt + 1) * 128],
                                        wo_sb[:, p, d * QSB:(d + 1) * QSB],
                                        start=(p == 0), stop=(p == NPAIR - 1))
                                nc.any.tensor_copy(
                                    ob[:, t2, d * QSB:(d + 1) * QSB], po)
                                yield
                        tt0 = 4 * i + 2 * tpair
                        if i == NCH - 1 and tpair == 1:
                            # tail: split so the first half overlaps the
                            # second half's evictions
                            for t2 in range(2):
                                nc.sync.dma_start(
                                    out=out.ap()[(tt0 + t2) * 128:
                                                 (tt0 + t2 + 1) * 128, :],
                                    in_=ob[:, t2, :])
                        else:
                            nc.sync.dma_start(
                                out=out.ap()[tt0 * 128:(tt0 + 2) * 128, :].rearrange(
                                    "(t p) d -> p t d", p=128),
                                in_=ob)

                # interleaved emission: phase1(c+1) + phase3(c-1) ride along
                # phase2(c)'s j-loop so PE fills ACT-wait gaps.
                import contextlib
                loop_cm = (tc.For_i(0, loop_n, 1, staggered_reset=staggered)
                           if loop_n else contextlib.nullcontext())
                with loop_cm:
                 for _u in range(unroll):
                  for _ in p1_units(0):
                    pass
                  for c in range(NCH):
                      extras = []
                      if c + 1 < NCH:
                          extras.append(p1_units(c + 1))
                      if c > 0:
                          extras.append(p3_units(c - 1))

                      def drain_extras(k):
                          n = 0
                          while extras and n < k:
                              try:
                                  next(extras[0])
                                  n += 1
                              except StopIteration:
                                  extras.pop(0)

                      n2 = 2 * (4 * c + 4) + 2
                      nx = 8 + (8 if c + 1 < NCH else 0)
                      per = max(1, -(-nx // n2))
                      for _ in p2_units(c):
                          drain_extras(per)
                      drain_extras(1000)
                  for _ in p3_units(NCH - 1):
                      pass

    nc.compile()
    return nc


def prepare_in_maps(x, Wq, bq_, Wk, bk_, Wv, bv_, Wo, bo_):
    x = np.asarray(x, np.float32)
    xtc = [np.ascontiguousarray(x[0].T), np.ascontiguousarray(x[1].T)]
    in_maps = []
    for c in range(NCORES):
        b = c // 4
        g = c % 4
        sl = slice(DC * g, DC * (g + 1))
        in_maps.append({
            "xt": xtc[b],
            "wq": np.ascontiguousarray(np.asarray(Wq, np.float32)[:, sl] * 0.125),
            "wk": np.ascontiguousarray(np.asarray(Wk, np.float32)[:, sl]),
            "wv": np.ascontiguousarray(np.asarray(Wv, np.float32)[:, sl]),
            "wo": np.ascontiguousarray(np.asarray(Wo, np.float32)[sl, :]),
            "bq": np.ascontiguousarray(np.asarray(bq_, np.float32)[sl] * 0.125),
            "bk": np.ascontiguousarray(np.asarray(bk_, np.float32)[sl]),
            "bv": np.ascontiguousarray(np.asarray(bv_, np.float32)[sl]),
        })
    return in_maps


_NC_CACHE = {}


def _get_nc():
    if "nc" not in _NC_CACHE:
        _NC_CACHE["nc"] = build()
    return _NC_CACHE["nc"]


def kernel(x, Wq, bq, Wk, bk, Wv, bv, Wo, bo, _trace=False):
    nc = _get_nc()
    in_maps = prepare_in_maps(x, Wq, bq, Wk, bk, Wv, bv, Wo, bo)
    res = bass_utils.run_bass_kernel_spmd(
        nc, in_maps, core_ids=list(range(NCORES)), trace=_trace)
    if _trace:
        _NC_CACHE["last_results"] = res
    partials = [res.results[c]["out"] for c in range(NCORES)]
    bo = np.asarray(bo, np.float32)
    full = np.stack([
        partials[0] + partials[1] + partials[2] + partials[3] + bo,
        partials[4] + partials[5] + partials[6] + partials[7] + bo,
    ]).astype(np.float32)
    return full
